# revision 39
# baseline (speedup 1.0000x reference)
"""BiMamba layer (fwd+bwd selective-scan mamba blocks + FFN) on 8 Trainium2
NeuronCores via Bass/Tile.

Sharding: data-parallel over batch - core i processes sample i (B=8).
Layout: channel-major [channel_partitions, time] on device; host pre-transposes
x and weights (bf16), device returns (d_model, L) f32, host transposes back.

v5 design (vs v2 baseline; same-regime HW: 6.9ms -> 5.35ms; the
axon/device speed drifts ~20% between sessions, so absolute numbers
vary: an earlier v4 checkpoint measured 4.98ms in a fast window):
- conv folded into in_proj (4 shifted-window matmuls on PE) - removes the
  all-DVE depthwise conv (scalar_tensor_tensor has NO fast DVE modes).
- scan tiles use T1=258 with TWO breaker columns so every data window and
  plane start is 4-byte aligned -> TT muls hit 2x_1p on HW (with T1=257 the
  odd plane stride forced half the planes to 1x; the cost model does not
  check alignment but HW does).
- dA decay powers w^n = exp(n*lnw) built as 16 ACT exp ops per d-block,
  emitted 2 d-slots ahead of their scan (dA_prep) so they never gate the
  DVE queue; lnw = ln(sigmoid(-(u+dt_b))) = -softplus(u+dt_b) on ACT.
- y = sum_n h*C reduce tree ENTIRELY on DVE (pool_levels=0). HW A/B
  showed GPSIMD/Pool shares its SBUF port with DVE: a Pool-heavy tree
  (sim-optimal) measures 7.99ms vs 4.98ms all-DVE. Keep Pool idle.
- h*C in place into the scan tile; reduce tree over t8 planes with the
  4->2->1 tail + yg/ys deferred 2 d-slots past the producing scan.
- g=lnw*xc emitted AFTER the previous dloop (front_g) so the DVE queue
  never waits on this chunk's ACT chain; xc*D via ACT Copy-with-scale and
  the chunk-state snapshot via ACT copy - removing even these tiny DVE
  ops measured -0.66ms (cross-engine-dep DVE ops cost far more on HW
  than their busy time).
- stage writes stay on the SP DMA queue: routing them via the ACT hwdge
  queue measured +1.7ms (triggers serialize the ACT sequencer).
- xproj emits one [80,T] tile laid out [dt(32)|B(16)|pad|C(16)]; B/C
  row-broadcast via PE selection matmuls; ACT downcast copies.
- 0.5 averaging folded into ln_f/ln_b gamma+beta on the host.
"""

import sys

sys.path.insert(0, "/opt/trn_rl_repo")

import numpy as np

import concourse.bass as bass
import concourse.mybir as mybir
import concourse.tile as tile

F32 = mybir.dt.float32
BF16 = mybir.dt.bfloat16
AF = mybir.ActivationFunctionType
ALU = mybir.AluOpType

D_MODEL = 512
D_FF = 2048
D_STATE = 16
D_CONV = 4
D_INNER = 1024
DT_RANK = 32
EPS = 1e-5

N_CORES = 8
L_FULL = 4096
T_CHUNK = 256

# ----------------------------------------------------------------------------
# walrus workaround: this compiler build rejects >1 semaphore wait per
# instruction. Hoist excess waits onto same-engine NoOps placed just before
# the instruction (engines execute their queue in order, so semantics hold).
# ----------------------------------------------------------------------------
_wait_ctr = [0]


def split_multi_waits(nc, max_waits=1):
    for f in nc.m.functions:
        for blk in f.blocks:
            insts = list(blk.instructions)
            out = []
            changed = False
            for inst in insts:
                si = inst.sync_info
                waits = list(si.on_wait) if si and si.on_wait else []
                if len(waits) > max_waits:
                    changed = True
                    extra, keep = waits[:-max_waits], waits[-max_waits:]
                    for w in extra:
                        _wait_ctr[0] += 1
                        nop = mybir.InstNoOp(name=f"I-waitsplit-{_wait_ctr[0]}")
                        nop.engine = inst.engine
                        nop.sync_info = mybir.SyncInfo(on_wait=[w], on_update=[])
                        out.append(nop)
                    si.on_wait = keep
                out.append(inst)
            if changed:
                blk.instructions = out


# ----------------------------------------------------------------------------
# device program builder
# ----------------------------------------------------------------------------
def build_program(L=L_FULL, T=T_CHUNK, n_cores=N_CORES, repeat=1,
                  pool_levels=2, conv_fold=True):
    C = L // T
    assert C * T == L
    ND = D_INNER // 128   # 8 d-blocks
    NM = D_MODEL // 128   # 4 k-tiles of d_model
    NF = D_FF // 128      # 16 m-tiles of d_ff

    nc = bass.Bass("TRN2", target_bir_lowering=False, debug=False,
                   num_devices=n_cores)

    def par(name, shape, out=False, dt=BF16):
        return nc.declare_dram_parameter(name, list(shape), dt, isOutput=out)

    xT = par("xT", (D_MODEL, L))
    outT = par("outT", (D_MODEL, L), out=True, dt=F32)
    W = {}
    for p in ("f", "b"):
        W[p] = dict(
            inwj=[par(f"{p}_inwj{j}T", (D_MODEL, D_INNER))
                  for j in range(D_CONV)],
            inwz=par(f"{p}_inwzT", (D_MODEL, D_INNER)),
            out_wT=par(f"{p}_out_wT", (D_INNER, D_MODEL)),
            xp_wT=par(f"{p}_xp_wT", (D_INNER, 80)),
            dt_wT=par(f"{p}_dt_wT", (DT_RANK, D_INNER)),
            conv_b=par(f"{p}_conv_b", (D_INNER, 1), dt=F32),
            dtb=par(f"{p}_dtb", (D_INNER, 1), dt=F32),
            D=par(f"{p}_D", (D_INNER, 1), dt=F32),
        )
    LN = {k: par(k, (D_MODEL, 1), dt=F32) for k in
          ("lnf_g", "lnf_b", "lnb_g", "lnb_b", "lnff_g", "lnff_b")}
    w1T = par("w1T", (D_MODEL, D_FF))
    b1 = par("b1", (D_FF, 1), dt=F32)
    w2T = par("w2T", (D_FF, D_MODEL))
    b2 = par("b2", (D_MODEL, 1), dt=F32)
    selbc = par("selbc", (80, 16 * 128))

    of_d = nc.dram_tensor("of_d", [D_MODEL, L], F32)
    ob_d = nc.dram_tensor("ob_d", [D_MODEL, L], F32)

    with tile.TileContext(nc) as tc:
        with tc.tile_pool(name="const", bufs=1) as cpool:
            ones_col = cpool.tile([128, 1], F32, tag="ones_col", name="ones_col")
            nc.vector.memset(ones_col[:], 1.0)
            ones_row = cpool.tile([1, 128], F32, tag="ones_row", name="ones_row")
            nc.vector.memset(ones_row[:], 1.0)
            eps_t = cpool.tile([1, 1], F32, tag="eps_t", name="eps_t")
            nc.vector.memset(eps_t[:], EPS)
            ones_bc = cpool.tile([80, 16 * 128], BF16, tag="ones_bc",
                                 name="ones_bc")
            nc.sync.dma_start(ones_bc[:], selbc[:])

            for _rep in range(repeat):
                for p, fwd, stage in (("f", True, of_d), ("b", False, ob_d)):
                    _direction(nc, tc, W[p], LN, xT, stage, fwd, p,
                               L, T, C, ND, NM, ones_col, ones_row, ones_bc,
                               eps_t, pool_levels)
                _ffn_phase(nc, tc, LN, w1T, b1, w2T, b2, of_d, ob_d, outT,
                           L, T, C, NM, NF, ones_col, ones_row, eps_t)

    return nc


def _load_weights(nc, wp, w, lng_name, lnb_name, LN, ND, NM):
    s = {}
    s["inwj"] = [[wp.tile([128, D_INNER], BF16, tag=f"inwj{j}_{k}",
                          name=f"inwj{j}_{k}") for k in range(NM)]
                 for j in range(D_CONV)]
    for j in range(D_CONV):
        for k in range(NM):
            nc.sync.dma_start(s["inwj"][j][k][:],
                              w["inwj"][j][128 * k:128 * (k + 1), :])
    s["inwz"] = [wp.tile([128, D_INNER], BF16, tag=f"inwz{k}", name=f"inwz{k}")
                 for k in range(NM)]
    for k in range(NM):
        nc.sync.dma_start(s["inwz"][k][:], w["inwz"][128 * k:128 * (k + 1), :])
    s["outw"] = [wp.tile([128, D_MODEL], BF16, tag=f"outw{k}", name=f"outw{k}")
                 for k in range(ND)]
    for k in range(ND):
        nc.sync.dma_start(s["outw"][k][:], w["out_wT"][128 * k:128 * (k + 1), :])
    s["xpw"] = [wp.tile([128, 80], BF16, tag=f"xpw{k}", name=f"xpw{k}")
                for k in range(ND)]
    for k in range(ND):
        nc.sync.dma_start(s["xpw"][k][:], w["xp_wT"][128 * k:128 * (k + 1), :])
    s["dtw"] = wp.tile([DT_RANK, D_INNER], BF16, tag="dtw", name="dtw")
    nc.sync.dma_start(s["dtw"][:], w["dt_wT"][:])
    for nm, key in (("convb", "conv_b"), ("dtb", "dtb"), ("Dp", "D")):
        s[nm] = [wp.tile([128, 1], F32, tag=f"{nm}{d}", name=f"{nm}{d}")
                 for d in range(ND)]
        for d in range(ND):
            nc.sync.dma_start(s[nm][d][:], w[key][128 * d:128 * (d + 1), :])
    s["lng"] = [wp.tile([128, 1], F32, tag=f"lng{k}", name=f"lng{k}") for k in range(NM)]
    s["lnb"] = [wp.tile([128, 1], F32, tag=f"lnb{k}", name=f"lnb{k}") for k in range(NM)]
    for k in range(NM):
        nc.sync.dma_start(s["lng"][k][:], LN[lng_name][128 * k:128 * (k + 1), :])
        nc.sync.dma_start(s["lnb"][k][:], LN[lnb_name][128 * k:128 * (k + 1), :])
    return s


def _layernorm(nc, ln_in, lng, lnb, psS, psM, smtmp, lnout_pool, ones_col,
               ones_row, eps_t, T, NM, tag, out_dt=BF16):
    """LN over the channel (partition) axis; ln_in: NM tiles [128,T] f32.
    Stats via PE ones-matmuls; normalize with bf16 fast-mode DVE ops."""
    # matmul outs need base partition 0/32/64: sum at p0, sq-sum at p32
    ps_sq = psS.tile([33, T], F32, tag="stat", name="stat")
    ps_s, ps_q = ps_sq[0:1, :], ps_sq[32:33, :]
    for k in range(NM):
        nc.tensor.matmul(ps_s, ones_col[:], ln_in[k][:],
                         start=(k == 0), stop=(k == NM - 1))
    sq = [None] * NM
    for k in range(NM):
        sq[k] = smtmp.tile([128, T], F32, tag="lnsq", name="lnsq", bufs=2)
        nc.scalar.square(sq[k][:], ln_in[k][:])
    for k in range(NM):
        nc.tensor.matmul(ps_q, ones_col[:], sq[k][:],
                         start=(k == 0), stop=(k == NM - 1))
    mu = smtmp.tile([1, T], F32, tag="mu", name="mu", bufs=1)
    nc.vector.tensor_scalar_mul(mu[:], ps_s, 1.0 / D_MODEL)
    m2 = smtmp.tile([1, T], F32, tag="m2", name="m2", bufs=1)
    nc.vector.tensor_scalar_mul(m2[:], ps_q, 1.0 / D_MODEL)
    mu2 = smtmp.tile([1, T], F32, tag="mu2", name="mu2", bufs=1)
    nc.vector.tensor_mul(mu2[:], mu[:], mu[:])
    var = smtmp.tile([1, T], F32, tag="var", name="var", bufs=1)
    nc.vector.tensor_sub(var[:], m2[:], mu2[:])
    # rstd = exp(-0.5*ln(var+eps))  (stays in the Ln/Exp ACT table)
    lnv = smtmp.tile([1, T], F32, tag="mu2", name="lnv", bufs=1)
    nc.scalar.activation(lnv[:], var[:], AF.Ln, bias=eps_t[:])
    rstd = smtmp.tile([1, T], F32, tag="m2", name="rstd", bufs=1)
    nc.scalar.activation(rstd[:], lnv[:], AF.Exp, scale=-0.5)
    ps_mr = psM.tile([128, 2, T], F32, tag="bcst", name="bcst")
    ps_mu, ps_rs = ps_mr[:, 0, :], ps_mr[:, 1, :]
    nc.tensor.matmul(ps_mu, ones_row[:], mu[:], start=True, stop=True)
    nc.tensor.matmul(ps_rs, ones_row[:], rstd[:], start=True, stop=True)
    rs16 = smtmp.tile([128, T], BF16, tag="rs16", name="rs16", bufs=1)
    nc.scalar.copy(rs16[:], ps_rs)
    outs = []
    for k in range(NM):
        t1 = smtmp.tile([128, T], BF16, tag="lt1", name="lt1", bufs=1)
        nc.vector.tensor_sub(t1[:], ln_in[k][:], ps_mu)
        t2 = smtmp.tile([128, T], BF16, tag="lt2", name="lt2", bufs=1)
        nc.vector.tensor_mul(t2[:], t1[:], rs16[:])
        o = lnout_pool.tile([128, T], out_dt, tag=tag)
        nc.vector.tensor_scalar(o[:], t2[:], lng[k][:], lnb[k][:],
                                op0=ALU.mult, op1=ALU.add)
        outs.append(o)
    return outs


def _direction(nc, tc, w, LN, xT, stage_d, fwd, p, L, T, C, ND, NM,
               ones_col, ones_row, ones_bc, eps_t, pool_levels):
    from contextlib import ExitStack
    T1 = T + 2
    # data window starts 4B-aligned (bf16): fwd cols [2,T+2), bwd cols [0,T)
    doff = 2 if fwd else 0
    boff = 0 if fwd else T          # first breaker column index
    with ExitStack() as ctx:
        wp = ctx.enter_context(tc.tile_pool(name=f"w_{p}", bufs=1))
        sw = _load_weights(nc, wp, w, f"ln{p}_g", f"ln{p}_b", LN, ND, NM)

        xk_pool = ctx.enter_context(tc.tile_pool(name=f"xk_{p}", bufs=12))
        tmp_pool = ctx.enter_context(tc.tile_pool(name=f"tmp_{p}", bufs=3))
        xc_pool = ctx.enter_context(tc.tile_pool(name=f"xc_{p}", bufs=16))
        zs_pool = ctx.enter_context(tc.tile_pool(name=f"zs_{p}", bufs=16))
        dbc_pool = ctx.enter_context(tc.tile_pool(name=f"dbc_{p}", bufs=1))
        wdt_pool = ctx.enter_context(tc.tile_pool(name=f"wdt_{p}", bufs=16))
        g_pool = ctx.enter_context(tc.tile_pool(name=f"g_{p}", bufs=8))
        rep_pool = ctx.enter_context(tc.tile_pool(name=f"rep_{p}", bufs=2))
        dA_pool = ctx.enter_context(tc.tile_pool(name=f"dA_{p}", bufs=3))
        b_pool = ctx.enter_context(tc.tile_pool(name=f"b_{p}", bufs=3))
        t8_pool = ctx.enter_context(tc.tile_pool(name=f"t8_{p}", bufs=2))
        st_pool = ctx.enter_context(tc.tile_pool(name=f"st_{p}", bufs=1))
        y_pool = ctx.enter_context(tc.tile_pool(name=f"y_{p}", bufs=3))
        ys_pool = ctx.enter_context(tc.tile_pool(name=f"ys_{p}", bufs=16))
        ln_pool = ctx.enter_context(tc.tile_pool(name=f"ln_{p}", bufs=4))
        lo_pool = ctx.enter_context(tc.tile_pool(name=f"lo_{p}", bufs=2))

        psA = ctx.enter_context(tc.tile_pool(name=f"psA_{p}", bufs=4, space="PSUM"))
        psB = ctx.enter_context(tc.tile_pool(name=f"psB_{p}", bufs=2, space="PSUM"))
        psS = ctx.enter_context(tc.tile_pool(name=f"psS_{p}", bufs=1, space="PSUM"))
        psM = ctx.enter_context(tc.tile_pool(name=f"psM_{p}", bufs=1, space="PSUM"))

        state_prev = [None] * ND

        def front(ci):
            """PE/ACT/Pool front-end for chunk index ci: x load, in_proj
            (conv folded), silu, xproj, B/C broadcast, dt+sigmoid+ln,
            g=lnw*xc, xc*=D."""
            j = ci if fwd else (C - 1 - ci)
            t0 = j * T
            # fwd: cols [t0-3, t0+T) -> tap jj reads xk[:, jj:jj+T]
            # bwd: cols [t0, t0+T+3) -> tap jj reads xk[:, 3-jj:3-jj+T]
            xk = []
            edge = (j == 0) if fwd else (j == C - 1)
            for k in range(NM):
                t = xk_pool.tile([128, T + 3], BF16, tag="xk", name="xk")
                if not edge:
                    src0 = t0 - 3 if fwd else t0
                    nc.sync.dma_start(t[:], xT[128 * k:128 * (k + 1),
                                                src0:src0 + T + 3])
                elif fwd:
                    nc.vector.memset(t[:, 0:3], 0.0)
                    nc.sync.dma_start(t[:, 3:T + 3],
                                      xT[128 * k:128 * (k + 1), t0:t0 + T])
                else:
                    nc.vector.memset(t[:, T:T + 3], 0.0)
                    nc.sync.dma_start(t[:, 0:T],
                                      xT[128 * k:128 * (k + 1), t0:t0 + T])
                xk.append(t)
            dcol = 3 if fwd else 0           # data window start in xk
            xkd = [t[:, dcol:dcol + T] for t in xk]

            # in_proj (+ conv via 4 shifted matmul taps) and silu
            xc_tiles = [None] * ND
            zs_tiles = [None] * ND
            for m in range(ND):
                ps = psA.tile([128, T], F32, tag="mm", name="mm")
                first = True
                for jj in range(D_CONV):
                    off = jj if fwd else (3 - jj)
                    for k in range(NM):
                        last = (jj == D_CONV - 1) and (k == NM - 1)
                        nc.tensor.matmul(
                            ps[:],
                            sw["inwj"][jj][k][:, 128 * m:128 * (m + 1)],
                            xk[k][:, off:off + T], start=first, stop=last)
                        first = False
                xc = xc_pool.tile([128, T], BF16, tag="xc", name="xc")
                nc.scalar.activation(xc[:], ps[:], AF.Silu,
                                     bias=sw["convb"][m][:])
                xc_tiles[m] = xc
                psz = psA.tile([128, T], F32, tag="mm", name="mm")
                for k in range(NM):
                    nc.tensor.matmul(psz[:],
                                     sw["inwz"][k][:, 128 * m:128 * (m + 1)],
                                     xkd[k], start=(k == 0), stop=(k == NM - 1))
                zs = zs_pool.tile([128, T], BF16, tag="zs", name="zs")
                nc.scalar.activation(zs[:], psz[:], AF.Silu)
                zs_tiles[m] = zs

            # xproj -> [dt(32) | B(16) | pad(16) | C(16)]
            psd = psA.tile([80, T], F32, tag="mm", name="mm")
            for k in range(ND):
                nc.tensor.matmul(psd[:], sw["xpw"][k][:], xc_tiles[k][:],
                                 start=(k == 0), stop=(k == ND - 1))
            dbc = dbc_pool.tile([80, T], BF16, tag="dbc", name="dbc")
            nc.scalar.copy(dbc[:], psd[:])

            # broadcast B,C rows across partitions (2 planes per psum tile)
            Brep = rep_pool.tile([128, D_STATE, T], BF16, tag="brep", name="brep")
            Crep = rep_pool.tile([128, D_STATE, T], BF16, tag="crep", name="crep")
            for half, src_base, dst in ((0, 32, Brep), (1, 64, Crep)):
                for q in range(D_STATE // 2):
                    pb = psB.tile([128, 2, T], F32, tag="bc", name="bc")
                    for e in range(2):
                        n = 2 * q + e
                        nc.tensor.matmul(
                            pb[:, e, :],
                            ones_bc[src_base:src_base + 16,
                                    128 * n:128 * (n + 1)],
                            dbc[src_base:src_base + 16, :],
                            start=True, stop=True)
                    # GPSIMD cannot access PSUM on HW: both halves on ACT
                    nc.scalar.copy(dst[:, 2 * q:2 * q + 2, :], pb[:])

            # dt matmul; lnw = ln(sigmoid(-(u+dt_b))) = -softplus(u+dt_b)
            # = -dt  (walrus has no Softplus act table; sigmoid+ln is the
            # HW-proven chain)
            wts = [None] * ND
            for d in range(ND):
                ps = psA.tile([128, T], F32, tag="mm", name="mm")
                nc.tensor.matmul(ps[:], sw["dtw"][:, 128 * d:128 * (d + 1)],
                                 dbc[0:DT_RANK, :], start=True, stop=True)
                wt = wdt_pool.tile([128, T], BF16, tag="w", name="w",
                                   bufs=4)
                nc.scalar.activation(wt[:], ps[:], AF.Sigmoid, scale=-1.0,
                                     bias=sw["dtb"][d][:])
                wts[d] = wt
            lnws = [None] * ND
            for d in range(ND):
                lnw = wdt_pool.tile([128, T], BF16, tag="lnw", name="lnw")
                nc.scalar.activation(lnw[:], wts[d][:], AF.Ln)
                lnws[d] = lnw
            return dict(xkd=xkd, xcd=xc_tiles, zs=zs_tiles, dts=lnws,
                        g=[None] * ND, dA=[None] * ND, ys=[None] * ND,
                        Brep=Brep, Crep=Crep, t0=t0)

        def front_g(fr):
            """DVE part of the front phase, emitted AFTER the previous
            dloop so the DVE queue never waits on this chunk's ACT chain."""
            xc_tiles = fr["xcd"]
            for d in range(ND):
                # g' = lnw*xc = -dt*xc; the sign lives in the negated
                # B weights (host negates xp_wT B-rows): g'*B_neg = dt*xc*B
                g_t = g_pool.tile([128, T], BF16, tag="g", name="g")
                nc.vector.tensor_mul(g_t[:], fr["dts"][d][:],
                                     xc_tiles[d][:])
                fr["g"][d] = g_t
                # after g, xc is only needed as xc*D (yg term): ACT
                # Copy-with-scale keeps it off the bottleneck DVE entirely
                nc.scalar.activation(xc_tiles[d][:], xc_tiles[d][:],
                                     AF.Copy, scale=sw["Dp"][d][:])

        def dA_prep(fr, dd):
            """Breaker memset + 16 ACT exp planes w^n = exp(n*lnw) for
            d-block dd; emitted ~2 d-slots ahead of its scan so the exps
            never gate the DVE queue."""
            dA = dA_pool.tile([128, D_STATE, T1], BF16, tag="dA", name="dA")
            nc.gpsimd.memset(dA[:, :, boff:boff + 2], 0.0)
            dt_t = fr["dts"][dd]
            for i in range(D_STATE):
                nc.scalar.activation(dA[:, i, doff:doff + T], dt_t[:],
                                     AF.Exp, scale=float(i + 1))
            fr["dA"][dd] = dA

        pend_q = []                      # (fr, d, y_t): Pool trees in flight

        def _flush_one():
            # deferred by two d-slots: Pool does tree levels 1-2 (16->8->4,
            # ~6.3us) under the next d-slots' DVE work; DVE finishes 4->1
            # here, by which time Pool's half is long done
            fr_p, dp, t8p = pend_q.pop(0)
            nc.vector.tensor_add(t8p[:, 0:2, :], t8p[:, 0:2, :],
                                 t8p[:, 2:4, :])
            y_t = y_pool.tile([128, T], BF16, tag="y", name="y")
            nc.vector.tensor_add(y_t[:], t8p[:, 0, :], t8p[:, 1, :])
            yg = y_pool.tile([128, T], BF16, tag="yg", name="yg", bufs=1)
            nc.vector.tensor_add(yg[:], y_t[:], fr_p["xcd"][dp][:])
            ys = ys_pool.tile([128, T], BF16, tag="ys", name="ys")
            nc.vector.tensor_mul(ys[:], yg[:], fr_p["zs"][dp][:])
            fr_p["ys"][dp] = ys

        def dloop(ci, fr, fr_next):
            """DVE/Pool scan pipeline for chunk ci using front-end results.
            Also preps dA planes 2 d-slots ahead (rolling into fr_next)."""
            g_tiles, Brep, Crep = fr["g"], fr["Brep"], fr["Crep"]

            for d in range(ND):
                if d + 2 < ND:
                    dA_prep(fr, d + 2)
                elif fr_next is not None:
                    dA_prep(fr_next, d + 2 - ND)
                dA = fr["dA"][d]

                # b = g*B; the two breaker cols carry the chunk-carry state
                # (first col killed by dA=0, second col = injected state)
                bt = b_pool.tile([128, D_STATE, T1], BF16, tag="b", name="b")
                gb = g_tiles[d][:].unsqueeze(1).broadcast_to([128, D_STATE, T])
                nc.vector.tensor_mul(bt[:, :, doff:doff + T], gb,
                                     Brep[:, :, :])
                if ci == 0:
                    nc.vector.memset(bt[:, :, boff:boff + 2], 0.0)
                else:
                    nc.scalar.copy(
                        bt[:, :, boff:boff + 2],
                        state_prev[d][:].unsqueeze(2).broadcast_to(
                            [128, D_STATE, 2]))
                flat_a = dA[:, :, :].rearrange("p n t -> p (n t)")
                flat_b = bt[:, :, :].rearrange("p n t -> p (n t)")
                if fwd:
                    nc.vector.tensor_tensor_scan(flat_b, flat_a, flat_b, 0.0,
                                                 op0=ALU.mult, op1=ALU.add)
                else:
                    nc.vector.tensor_tensor_scan(flat_b[:, ::-1],
                                                 flat_a[:, ::-1],
                                                 flat_b[:, ::-1], 0.0,
                                                 op0=ALU.mult, op1=ALU.add)
                stt = st_pool.tile([128, D_STATE], F32, tag=f"st{d}",
                                   name=f"st{d}")
                nc.scalar.copy(stt[:], bt[:, :, (T1 - 1) if fwd else 0])
                state_prev[d] = stt
                # flush before hC: the deferred d-2 flush reads its t8
                # BEFORE Pool's tree(d) rotates onto that buffer
                while len(pend_q) > 1:
                    _flush_one()
                # y = sum_n h*C : h*C in place into the scan tile (DVE 2x),
                # then pool_levels tree levels on Pool, rest in the flush
                hC = bt[:, :, doff:doff + T]
                nc.vector.tensor_mul(hC, hC, Crep[:, :, :])
                t8 = t8_pool.tile([128, 8, T], BF16, tag="t8", name="t8")
                e1 = nc.gpsimd if pool_levels >= 1 else nc.vector
                e2 = nc.gpsimd if pool_levels >= 2 else nc.vector
                e1.tensor_add(t8[:, :, :],
                              bt[:, 0:8, doff:doff + T],
                              bt[:, 8:16, doff:doff + T])
                e2.tensor_add(t8[:, 0:4, :], t8[:, 0:4, :],
                              t8[:, 4:8, :])
                pend_q.append((fr, d, t8))

        def tail(ci, fr):
            """out_proj + residual + layernorm + stage write for chunk ci."""
            t0 = fr["t0"]
            ln_in = [None] * NM
            for m in range(NM):
                ps = psA.tile([128, T], F32, tag="mm", name="mm")
                for k in range(ND):
                    nc.tensor.matmul(ps[:],
                                     sw["outw"][k][:, 128 * m:128 * (m + 1)],
                                     fr["ys"][k][:], start=(k == 0),
                                     stop=(k == ND - 1))
                li = ln_pool.tile([128, T], F32, tag="lnin", name="lnin")
                nc.vector.tensor_add(li[:], fr["xkd"][m], ps[:])
                ln_in[m] = li
            outs = _layernorm(nc, ln_in, sw["lng"], sw["lnb"], psS, psM,
                              tmp_pool, lo_pool, ones_col, ones_row, eps_t,
                              T, NM, tag="lo", out_dt=F32)
            for m in range(NM):
                nc.sync.dma_start(stage_d[128 * m:128 * (m + 1), t0:t0 + T],
                                  outs[m][:])

        # software pipeline: front(ci+1) ahead of dloop(ci); the DVE part
        # of front (g, xc*D) is emitted after dloop(ci) so the DVE queue
        # never blocks on front(ci+1)'s ACT chain; dA preps roll 2 d-slots
        # ahead across chunk boundaries; tail lags one chunk
        frs = {0: front(0)}
        front_g(frs[0])
        dA_prep(frs[0], 0)
        dA_prep(frs[0], 1)
        pend = None                      # (ci, fr) awaiting tail
        for ci in range(C):
            if ci + 1 < C:
                frs[ci + 1] = front(ci + 1)
            fr_next = frs.get(ci + 1)
            dloop(ci, frs[ci], fr_next)
            if fr_next is not None:
                front_g(fr_next)
            if pend is not None:
                tail(*pend)
            pend = (ci, frs.pop(ci))
        while pend_q:
            _flush_one()
        tail(*pend)


def _ffn_phase(nc, tc, LN, w1T, b1, w2T, b2, of_d, ob_d, outT,
               L, T, C, NM, NF, ones_col, ones_row, eps_t):
    from contextlib import ExitStack
    with ExitStack() as ctx:
        wp = ctx.enter_context(tc.tile_pool(name="w_ffn", bufs=1))
        w1s = [wp.tile([128, D_FF], BF16, tag=f"w1_{k}", name=f"w1_{k}") for k in range(NM)]
        for k in range(NM):
            nc.sync.dma_start(w1s[k][:], w1T[128 * k:128 * (k + 1), :])
        w2s = [wp.tile([128, D_MODEL], BF16, tag=f"w2_{k}", name=f"w2_{k}") for k in range(NF)]
        for k in range(NF):
            nc.sync.dma_start(w2s[k][:], w2T[128 * k:128 * (k + 1), :])
        b1s = [wp.tile([128, 1], F32, tag=f"b1_{m}", name=f"b1_{m}") for m in range(NF)]
        for m in range(NF):
            nc.sync.dma_start(b1s[m][:], b1[128 * m:128 * (m + 1), :])
        b2s = [wp.tile([128, 1], F32, tag=f"b2_{m}", name=f"b2_{m}") for m in range(NM)]
        for m in range(NM):
            nc.sync.dma_start(b2s[m][:], b2[128 * m:128 * (m + 1), :])
        lng = [wp.tile([128, 1], F32, tag=f"lng{k}", name=f"lng{k}") for k in range(NM)]
        lnb = [wp.tile([128, 1], F32, tag=f"lnb{k}", name=f"lnb{k}") for k in range(NM)]
        for k in range(NM):
            nc.sync.dma_start(lng[k][:], LN["lnff_g"][128 * k:128 * (k + 1), :])
            nc.sync.dma_start(lnb[k][:], LN["lnff_b"][128 * k:128 * (k + 1), :])

        io_pool = ctx.enter_context(tc.tile_pool(name="ffn_io", bufs=10))
        h_pool = ctx.enter_context(tc.tile_pool(name="ffn_h", bufs=5))
        h1_pool = ctx.enter_context(tc.tile_pool(name="ffn_h1", bufs=17))
        tmp_pool = ctx.enter_context(tc.tile_pool(name="ffn_tmp", bufs=3))
        ln_pool = ctx.enter_context(tc.tile_pool(name="ffn_ln", bufs=5))
        lo_pool = ctx.enter_context(tc.tile_pool(name="ffn_lo", bufs=4))
        psA = ctx.enter_context(tc.tile_pool(name="ffn_psA", bufs=6, space="PSUM"))
        psS = ctx.enter_context(tc.tile_pool(name="ffn_psS", bufs=1, space="PSUM"))
        psM = ctx.enter_context(tc.tile_pool(name="ffn_psM", bufs=1, space="PSUM"))

        for ci in range(C):
            t0 = ci * T
            hk = [None] * NM
            hfk = [None] * NM
            for k in range(NM):
                a = io_pool.tile([128, T], F32, tag="of", name="of")
                nc.sync.dma_start(a[:], of_d[128 * k:128 * (k + 1), t0:t0 + T])
                bb = io_pool.tile([128, T], F32, tag="ob", name="ob")
                nc.sync.dma_start(bb[:], ob_d[128 * k:128 * (k + 1), t0:t0 + T])
                # 0.5 scaling folded into ln_f/ln_b params on the host.
                # h kept in two precisions: f32 for the residual into the
                # final LN, bf16 for the w1 matmul operand.
                hf = io_pool.tile([128, T], F32, tag="hf", name="hf")
                nc.vector.tensor_add(hf[:], a[:], bb[:])
                h = h_pool.tile([128, T], BF16, tag="h", name="h")
                nc.vector.tensor_copy(h[:], hf[:])
                hk[k] = h
                hfk[k] = hf
            h1 = [None] * NF
            for m in range(NF):
                ps = psA.tile([128, T], F32, tag="mm", name="mm")
                for k in range(NM):
                    nc.tensor.matmul(ps[:], w1s[k][:, 128 * m:128 * (m + 1)],
                                     hk[k][:], start=(k == 0),
                                     stop=(k == NM - 1))
                t = h1_pool.tile([128, T], BF16, tag="h1", name="h1")
                nc.scalar.activation(t[:], ps[:], AF.Gelu_apprx_tanh,
                                     bias=b1s[m][:])
                h1[m] = t
            ln_in = [None] * NM
            for m in range(NM):
                ps = psA.tile([128, T], F32, tag="mm", name="mm")
                for k in range(NF):
                    nc.tensor.matmul(ps[:], w2s[k][:, 128 * m:128 * (m + 1)],
                                     h1[k][:], start=(k == 0),
                                     stop=(k == NF - 1))
                li = ln_pool.tile([128, T], F32, tag="lnin", name="lnin")
                # (ps + b2) + h
                nc.vector.scalar_tensor_tensor(li[:], ps[:], b2s[m][:],
                                               hfk[m][:],
                                               op0=ALU.add, op1=ALU.add)
                ln_in[m] = li
            outs = _layernorm(nc, ln_in, lng, lnb, psS, psM, tmp_pool,
                              lo_pool, ones_col, ones_row, eps_t, T, NM,
                              tag="lo", out_dt=F32)
            for m in range(NM):
                nc.sync.dma_start(outT[128 * m:128 * (m + 1), t0:t0 + T],
                                  outs[m][:])


# ----------------------------------------------------------------------------
# host side: input packing, cached jitted runner
# ----------------------------------------------------------------------------
def pack_inputs(inputs, n_cores=N_CORES):
    """Host-side layout prep: transposes, bf16 casts, conv folded into
    in_proj (W_j = in_w_x * conv_w[:,j]), 0.5 avg folded into ln_f/ln_b."""
    import ml_dtypes
    f32 = np.float32
    bf16 = ml_dtypes.bfloat16

    def tb(a):
        return np.ascontiguousarray(np.asarray(a, f32).T).astype(bf16)

    shared = {}
    for p in ("f", "b"):
        in_w = np.asarray(inputs[f"{p}_in_w"], f32)        # (2048, 512)
        conv_w = np.asarray(inputs[f"{p}_conv_w"], f32)    # (1024, 4)
        in_w_x, in_w_z = in_w[:D_INNER], in_w[D_INNER:]
        for j in range(D_CONV):
            wj = in_w_x * conv_w[:, j:j + 1]               # (1024, 512)
            shared[f"{p}_inwj{j}T"] = tb(wj)
        shared[f"{p}_inwzT"] = tb(in_w_z)
        shared[f"{p}_out_wT"] = tb(inputs[f"{p}_out_w"])
        xp = np.asarray(inputs[f"{p}_xproj_w"], f32)       # (64, 1024)
        # layout [dt(32) | B(16) | pad(16) | C(16)] for PE bases 0/32/64
        xp_pad = np.zeros((80, D_INNER), f32)
        xp_pad[0:48] = xp[0:48]                            # dt-lowrank + B
        xp_pad[32:48] *= -1.0     # sign of g = lnw*xc folded into B
        xp_pad[64:80] = xp[48:64]                          # C
        shared[f"{p}_xp_wT"] = tb(xp_pad)
        shared[f"{p}_dt_wT"] = tb(inputs[f"{p}_dt_w"])
        shared[f"{p}_conv_b"] = np.asarray(inputs[f"{p}_conv_b"], f32).reshape(-1, 1)
        shared[f"{p}_dtb"] = -np.asarray(inputs[f"{p}_dt_b"], f32).reshape(-1, 1)
        shared[f"{p}_D"] = np.asarray(inputs[f"{p}_D"], f32).reshape(-1, 1)
    for src, dst, half in (("ln_f_g", "lnf_g", True), ("ln_f_b", "lnf_b", True),
                           ("ln_b_g", "lnb_g", True), ("ln_b_b", "lnb_b", True),
                           ("ln_ff_g", "lnff_g", False), ("ln_ff_b", "lnff_b", False)):
        v = np.asarray(inputs[src], f32).reshape(-1, 1)
        shared[dst] = v * (0.5 if half else 1.0)
    shared["w1T"] = tb(inputs["ffn_w1"])
    shared["b1"] = np.asarray(inputs["ffn_b1"], f32).reshape(-1, 1)
    shared["w2T"] = tb(inputs["ffn_w2"])
    shared["b2"] = np.asarray(inputs["ffn_b2"], f32).reshape(-1, 1)
    sel = np.zeros((80, 16 * 128), f32)
    for k in range(D_STATE):
        sel[32 + k, 128 * k:128 * (k + 1)] = 1.0
        sel[64 + k, 128 * k:128 * (k + 1)] = 1.0
    shared["selbc"] = sel.astype(bf16)

    x = np.asarray(inputs["x"], f32)
    in_maps = []
    for i in range(n_cores):
        m = dict(shared)
        m["xT"] = np.ascontiguousarray(x[i].T).astype(bf16)
        in_maps.append(m)
    return in_maps


_RUNNER = {}


def make_runner(**build_kwargs):
    import jax
    import jax.numpy as jnp
    from jax.experimental.shard_map import shard_map
    from jax.sharding import Mesh, NamedSharding, PartitionSpec
    from concourse import bass2jax

    nc = build_program(**build_kwargs)
    split_multi_waits(nc)
    bass2jax.install_neuronx_cc_hook()

    partition_name = (nc.partition_id_tensor.name
                      if nc.partition_id_tensor else None)
    in_names, out_names, out_avals, zero_shapes = [], [], [], []
    for alloc in nc.m.functions[0].allocations:
        if not isinstance(alloc, mybir.MemoryLocationSet):
            continue
        name = alloc.memorylocations[0].name
        if alloc.kind == "ExternalInput":
            if name != partition_name:
                in_names.append(name)
        elif alloc.kind == "ExternalOutput":
            shape = tuple(alloc.tensor_shape)
            dtype = mybir.dt.np(alloc.dtype)
            out_names.append(name)
            out_avals.append(jax.core.ShapedArray(shape, dtype))
            zero_shapes.append((shape, dtype))
    n_params = len(in_names)
    all_in_names = list(in_names) + list(out_names)
    if partition_name is not None:
        all_in_names.append(partition_name)

    def _body(*args):
        operands = list(args)
        if partition_name is not None:
            operands.append(bass2jax.partition_id_tensor())
        outs = bass2jax._bass_exec_p.bind(
            *operands,
            out_avals=tuple(out_avals),
            in_names=tuple(all_in_names),
            out_names=tuple(out_names),
            lowering_input_output_aliases=(),
            sim_require_finite=True,
            sim_require_nnan=True,
            nc=nc,
        )
        return tuple(outs)

    devices = jax.devices()[:N_CORES]
    mesh = Mesh(np.asarray(devices), ("core",))
    n_outs = len(out_avals)
    in_specs = (PartitionSpec("core"),) * (n_params + n_outs)
    out_specs = (PartitionSpec("core"),) * n_outs
    donate = tuple(range(n_params, n_params + n_outs))
    sharded = jax.jit(
        shard_map(_body, mesh=mesh, in_specs=in_specs, out_specs=out_specs,
                  check_rep=False),
        donate_argnums=donate, keep_unused=True)

    sh = NamedSharding(mesh, PartitionSpec("core"))

    def make_zeros():
        return tuple(
            jnp.zeros((N_CORES * s[0],) + tuple(s[1:]), d)
            for s, d in zero_shapes)

    zeros_fn = jax.jit(make_zeros, out_shardings=(sh,) * n_outs)

    return dict(
        fn=sharded, in_names=in_names, out_names=out_names,
        out_avals=out_avals, zeros_fn=zeros_fn, mesh=mesh, sh=sh, jnp=jnp,
        jax=jax)


CONV_FOLD = True
POOL_LEVELS = 0
BEST_CONFIG = dict(conv_fold=CONV_FOLD, pool_levels=POOL_LEVELS)


def _get_runner():
    if not _RUNNER:
        _RUNNER.update(make_runner(**BEST_CONFIG))
    return _RUNNER


def _device_inputs(in_maps, r=None):
    import jax
    r = r or _get_runner()
    concat = [np.concatenate([in_maps[c][n] for c in range(N_CORES)], axis=0)
              for n in r["in_names"]]
    return [jax.device_put(a, r["sh"]) for a in concat]


def _run_once(dev_in, r=None):
    r = r or _get_runner()
    zeros = r["zeros_fn"]()
    outs = r["fn"](*dev_in, *zeros)
    return outs


def kernel(**inputs):
    r = _get_runner()
    in_maps = pack_inputs(inputs)
    dev_in = _device_inputs(in_maps)
    outs = _run_once(dev_in)
    outT = np.asarray(outs[r["out_names"].index("outT")])
    outT = outT.reshape(N_CORES, D_MODEL, L_FULL)
    out = np.ascontiguousarray(np.transpose(outT, (0, 2, 1)).astype(np.float32))
    return out
